# revision 12
# baseline (speedup 1.0000x reference)
"""Trainium2 Bass kernel for nn_BinLinear: out = x @ where(clip(w,-1,1) >= 0, 1, -1).

Since clipping to [-1, 1] preserves sign, the binarized weight is exactly
where(w >= 0, +1, -1), so the kernel computes out = x @ sign01(w) as a dense
matmul on the TensorEngine.

Distribution: x is sharded row-wise across the 8 NeuronCores (1024 rows each);
the binarized w streams through each core once.  Each core computes an
independent [1024, 4096] output shard; the host concatenates shards (no
device collectives needed).

Precision/layout variants (BL_VARIANT env var):
  f16x1  - x cast to fp16, single matmul pass.            ~2e-4 rel err
  f16x2  - x split hi/lo fp16, two accumulating passes.   ~1e-7 rel err
  bf16x2 - x split hi/lo bf16, two accumulating passes.   ~3e-6 rel err
  f32r   - x kept fp32, matmul in float32r (tf32-like).   ~1e-4 rel err
"""

import os
import sys

for _p in ("/opt/trn_rl_repo", "/root/.axon_site/_ro/trn_rl_repo"):
    if os.path.isdir(_p) and _p not in sys.path:
        sys.path.append(_p)

import numpy as np
import ml_dtypes

import concourse.bacc as bacc
import concourse.mybir as mybir
from concourse.tile import TileContext
from concourse.bass_utils import run_bass_kernel_spmd

P = 128
NCORES = 8
B_FULL, K_DIM, N_FULL = 8192, 4096, 4096
M_CORE = B_FULL // NCORES  # 1024

VARIANT = os.environ.get("BL_VARIANT", "f16x1")

_NC_CACHE = {}


def _build_xstat(n_passes, mmdt, panel, repeat=1):
    """Variant A: stationary = x^T tiles, moving = w panels, out natural [M, N].

    Per-core inputs: xt_hi [K, M_CORE] (+ xt_lo), wb [K, N_FULL] in mmdt.
    Output: out [M_CORE, N_FULL] fp32.
    repeat>1 wraps the compute in a hardware loop (for benchmarking).
    """
    import contextlib

    dt = mybir.dt
    KC = K_DIM // P
    MT = M_CORE // P
    NPAN = N_FULL // panel

    nc = bacc.Bacc("TRN2")
    xt_parts = [
        nc.dram_tensor(
            "xt_hi" if i == 0 else "xt_lo", [K_DIM, M_CORE], mmdt, kind="ExternalInput"
        )
        for i in range(n_passes)
    ]
    # w pre-packed on host so each (panel, kc-pair) DMA reads 2*panel
    # contiguous elements per partition (2-4KB lines instead of 0.5-1KB).
    wb_d = nc.dram_tensor(
        "wb", [KC // 2, P, NPAN, 2 * panel], mmdt, kind="ExternalInput"
    )
    out_d = nc.dram_tensor("out", [M_CORE, N_FULL], dt.float32, kind="ExternalOutput")

    with TileContext(nc) as tc:
        with (
            tc.tile_pool(name="xpool", bufs=1) as xpool,
            tc.tile_pool(name="wpool", bufs=2) as wpool,
            tc.tile_pool(name="opool", bufs=3) as opool,
            tc.tile_pool(name="pspool", bufs=8, space="PSUM") as pspool,
        ):
            xs = []
            xt_rs = []
            for i, xt_d in enumerate(xt_parts):
                xt_rs.append(xt_d.rearrange("(kc p) m -> p kc m", p=P))
                xs.append(xpool.tile([P, KC, M_CORE], mmdt, name=f"x{i}"))

            loop_cm = (
                tc.For_i(
                    0,
                    repeat,
                    1,
                    hint_engines=(
                        mybir.EngineType.PE,
                        mybir.EngineType.SP,
                        mybir.EngineType.DVE,
                    ),
                    name="rep",
                )
                if repeat > 1
                else contextlib.nullcontext()
            )
            def evict(ps, mt, ip):
                ot = opool.tile([P, panel], dt.float32, name="ot")
                nc.vector.tensor_copy(ot[:], ps[:])
                nc.sync.dma_start(
                    out=out_d[mt * P : (mt + 1) * P, ip * panel : (ip + 1) * panel],
                    in_=ot[:],
                )

            # x slab loads interleaved with panel-0 w loads so panel-0
            # compute (kc-outer, all 8 psum banks) tracks slab arrival.
            with loop_cm:
                # panel 0: kc-outer / mt-inner
                wt0 = wpool.tile([P, KC, panel], mmdt, name="wt")
                for kc2 in range(KC // 2):
                    nc.sync.dma_start(
                        out=wt0[:, 2 * kc2 : 2 * kc2 + 2, :], in_=wb_d[kc2, :, 0, :]
                    )
                    for i in range(n_passes):
                        nc.sync.dma_start(
                            out=xs[i][:, 2 * kc2, :], in_=xt_rs[i][:, 2 * kc2, :]
                        )
                        nc.sync.dma_start(
                            out=xs[i][:, 2 * kc2 + 1, :],
                            in_=xt_rs[i][:, 2 * kc2 + 1, :],
                        )
                pss = [
                    pspool.tile([P, panel], dt.float32, name="ps") for _ in range(MT)
                ]
                for kc in range(KC):
                    for mt in range(MT):
                        for ipass in range(n_passes):
                            nc.tensor.matmul(
                                pss[mt][:],
                                lhsT=xs[ipass][:, kc, mt * P : (mt + 1) * P],
                                rhs=wt0[:, kc, :],
                                start=(kc == 0 and ipass == 0),
                                stop=(kc == KC - 1 and ipass == n_passes - 1),
                                skip_group_check=True,
                            )
                # prefetch panel 1 BEFORE panel 0's evictions so its DMAs sit
                # ahead of the output DMAs in the HWDGE queues
                wts = {0: wt0}
                if NPAN > 1:
                    wts[1] = wpool.tile([P, KC, panel], mmdt, name="wt")
                    for kc2 in range(KC // 2):
                        nc.sync.dma_start(
                            out=wts[1][:, 2 * kc2 : 2 * kc2 + 2, :],
                            in_=wb_d[kc2, :, 1, :],
                        )
                for mt in range(MT):
                    evict(pss[mt], mt, 0)

                # panels 1..NPAN-1: mt-outer, prefetching panel ip+1 first
                for ip in range(1, NPAN):
                    wt = wts.pop(ip)
                    if ip + 1 < NPAN:
                        wts[ip + 1] = wpool.tile([P, KC, panel], mmdt, name="wt")
                        for kc2 in range(KC // 2):
                            nc.sync.dma_start(
                                out=wts[ip + 1][:, 2 * kc2 : 2 * kc2 + 2, :],
                                in_=wb_d[kc2, :, ip + 1, :],
                            )
                    for mt in range(MT):
                        ps = pspool.tile([P, panel], dt.float32, name="ps")
                        for kc in range(KC):
                            for ipass in range(n_passes):
                                nc.tensor.matmul(
                                    ps[:],
                                    lhsT=xs[ipass][:, kc, mt * P : (mt + 1) * P],
                                    rhs=wt[:, kc, :],
                                    start=(kc == 0 and ipass == 0),
                                    stop=(kc == KC - 1 and ipass == n_passes - 1),
                                )
                        evict(ps, mt, ip)
    nc.compile()
    return nc


def _build_wstat_f32r(bchunk=512, repeat=1):
    """Variant B (f32r): stationary = w column tiles, moving = resident x^T,
    out transposed [N_FULL, M_CORE].

    Per-core inputs: xt [K, M_CORE] fp32, wb [K, N_FULL] fp32.
    Output: out_t [N_FULL, M_CORE] fp32 (host transposes back).
    """
    dt = mybir.dt
    KC = K_DIM // P
    JT = N_FULL // P  # 32 stationary column tiles of w
    BC = M_CORE // bchunk  # moving chunks of x

    nc = bacc.Bacc("TRN2")
    xt_d = nc.dram_tensor("xt", [K_DIM, M_CORE], dt.float32r, kind="ExternalInput")
    wb_d = nc.dram_tensor("wb", [K_DIM, N_FULL], dt.float32r, kind="ExternalInput")
    out_d = nc.dram_tensor("out", [N_FULL, M_CORE], dt.float32, kind="ExternalOutput")

    with TileContext(nc) as tc:
        with (
            tc.tile_pool(name="xpool", bufs=1) as xpool,
            tc.tile_pool(name="wpool", bufs=3) as wpool,
            tc.tile_pool(name="opool", bufs=3) as opool,
            tc.tile_pool(name="pspool", bufs=4, space="PSUM") as pspool,
        ):
            xt_r = xt_d.rearrange("(kc p) m -> p kc m", p=P)
            xtile = xpool.tile([P, KC, M_CORE], dt.float32r, name="xres")
            for half in range(2):  # split load so compute starts early
                h = KC // 2
                nc.sync.dma_start(
                    out=xtile[:, half * h : (half + 1) * h, :],
                    in_=xt_r[:, half * h : (half + 1) * h, :],
                )

            wb_r = wb_d.rearrange("(kc p) n -> p kc n", p=P)

            import contextlib

            loop_cm = (
                tc.For_i(
                    0,
                    repeat,
                    1,
                    hint_engines=(
                        mybir.EngineType.PE,
                        mybir.EngineType.SP,
                        mybir.EngineType.DVE,
                    ),
                    name="rep",
                )
                if repeat > 1
                else contextlib.nullcontext()
            )
            with loop_cm:
                _body_wstat(nc, tc, wpool, opool, pspool, wb_r, xtile, out_d, KC, JT, BC, bchunk)
    nc.compile()
    return nc


def _body_wstat(nc, tc, wpool, opool, pspool, wb_r, xtile, out_d, KC, JT, BC, bchunk):
    dt = mybir.dt
    for jt in range(JT):
        wt = wpool.tile([P, KC, P], dt.float32r, name="wt")
        for kc in range(KC):
            nc.sync.dma_start(
                out=wt[:, kc, :],
                in_=wb_r[:, kc, jt * P : (jt + 1) * P],
            )
        pss = []
        for bc in range(BC):
            ps = pspool.tile([P, bchunk], dt.float32, name="ps")
            pss.append(ps)
        for kc in range(KC):
            for bc in range(BC):
                nc.tensor.matmul(
                    pss[bc][:],
                    lhsT=wt[:, kc, :],
                    rhs=xtile[:, kc, bc * bchunk : (bc + 1) * bchunk],
                    start=(kc == 0),
                    stop=(kc == KC - 1),
                    skip_group_check=True,
                )
        for bc in range(BC):
            ot = opool.tile([P, bchunk], dt.float32, name="ot")
            nc.vector.tensor_copy(ot[:], pss[bc][:])
            nc.sync.dma_start(
                out=out_d[jt * P : (jt + 1) * P, bc * bchunk : (bc + 1) * bchunk],
                in_=ot[:],
            )


def get_nc(variant=None, repeat=1):
    variant = variant or VARIANT
    key = (variant, repeat)
    if key not in _NC_CACHE:
        if variant == "f16x1":
            _NC_CACHE[key] = _build_xstat(1, mybir.dt.float16, 512, repeat)
        elif variant == "f16x2":
            _NC_CACHE[key] = _build_xstat(2, mybir.dt.float16, 256, repeat)
        elif variant == "bf16x2":
            _NC_CACHE[key] = _build_xstat(2, mybir.dt.bfloat16, 256, repeat)
        elif variant == "f32r":
            _NC_CACHE[key] = _build_wstat_f32r(512, repeat)
        else:
            raise ValueError(f"unknown variant {variant}")
    return _NC_CACHE[key]


def prep_in_maps(x, w, variant=None):
    """Host-side prep: binarize w, transpose/cast/split x, build per-core maps."""
    variant = variant or VARIANT
    x = np.ascontiguousarray(x, dtype=np.float32)
    wb = np.where(np.asarray(w) >= 0, np.float32(1.0), np.float32(-1.0))
    xt = np.ascontiguousarray(x.T)  # [K, B]

    if variant == "f32r":
        in_maps = []
        for c in range(NCORES):
            sl = slice(c * M_CORE, (c + 1) * M_CORE)
            in_maps.append({"xt": np.ascontiguousarray(xt[:, sl]), "wb": wb})
        return in_maps

    npdt = {"f16x1": np.float16, "f16x2": np.float16, "bf16x2": ml_dtypes.bfloat16}[
        variant
    ]
    n_passes = 1 if variant == "f16x1" else 2
    panel = 512 if variant == "f16x1" else 256
    KC, NPAN = K_DIM // P, N_FULL // panel
    xt_hi = xt.astype(npdt)
    wb16 = np.ascontiguousarray(
        wb.astype(npdt)
        .reshape(KC // 2, 2, P, NPAN, panel)
        .transpose(0, 2, 3, 1, 4)
        .reshape(KC // 2, P, NPAN, 2 * panel)
    )
    if n_passes == 2:
        xt_lo = (xt - xt_hi.astype(np.float32)).astype(npdt)

    in_maps = []
    for c in range(NCORES):
        sl = slice(c * M_CORE, (c + 1) * M_CORE)
        m = {"xt_hi": np.ascontiguousarray(xt_hi[:, sl]), "wb": wb16}
        if n_passes == 2:
            m["xt_lo"] = np.ascontiguousarray(xt_lo[:, sl])
        in_maps.append(m)
    return in_maps


def gather_out(results, variant=None):
    variant = variant or VARIANT
    if variant == "f32r":
        return np.concatenate(
            [np.asarray(results[c]["out"]).T for c in range(NCORES)], axis=0
        )
    return np.concatenate([np.asarray(results[c]["out"]) for c in range(NCORES)], axis=0)


def kernel(x, w):
    """Full inputs in, full output out.  x [8192, 4096] f32, w [4096, 4096] f32."""
    assert x.shape == (B_FULL, K_DIM) and w.shape == (K_DIM, N_FULL)
    nc = get_nc()
    in_maps = prep_in_maps(x, w)
    res = run_bass_kernel_spmd(nc, in_maps, core_ids=list(range(NCORES)))
    out = gather_out(res.results)
    return np.ascontiguousarray(out, dtype=np.float32)
